# revision 50
# baseline (speedup 1.0000x reference)
"""Trainium2 Bass kernel for CayleyStringPE (RoPE + Cayley orthogonal mix).

Math: out = C @ rope(x) per token, where C = (I-S)(I+S)^{-1} is a fixed
128x128 orthogonal matrix (Cayley transform of the skew-symmetric S built
from s_params), and rope applies interleaved-pair rotations by angle
pos[t]*freqs[i].

Device formulation: rope(x)_t = x_t*c_t + P x_t * s_t with P the fixed
pair-swap-sign matrix and c_t/s_t the duplicated cos/sin vectors, so

    out_t = A @ (x_t * c_t) + Bm @ (x_t * s_t),   A = C,  Bm = C @ P

i.e. two 128x128 matmuls per token tile plus two elementwise multiplies.
No cross-partition shuffles on device.

Precision: fp16 end-to-end (inputs, trig tables, weights, outputs) with
f32 PSUM accumulation.

Sharding: sequence-parallel across 8 cores (positions split 8 x 1024, all
batches on every core). cos/sin tables are per-core (128 x 1024) and reused
across the 8 batches. A/Bm replicated. No collectives.

Schedule: the whole per-core input stream (4 MiB) is brought into SBUF by
a handful of up-front DMAs with no buffer reuse, so the SP DMA queue runs
wall-to-wall with zero dependency stalls. Compute (DVE cos/sin muls, PE
matmul blocks of 2048 cols, ACT/GPSIMD PSUM drains) chases the input
stream; out-DMAs are dispatched from the ACT hardware queue and the
GPSIMD software queue so they never serialize behind queued input DMAs.
"""

import sys

import numpy as np

for _p in ("/opt/trn_rl_repo", "/opt/pypackages"):
    if _p not in sys.path:
        sys.path.insert(0, _p)

B, N, D = 8, 8192, 128
NCORES = 8
NSH = N // NCORES          # positions per core
TOK = B * NSH              # tokens per core
FTOK = 2 * TOK             # fused q|k stream columns per core
BLK = 2048                 # compute block (TT + matmul + drain granularity)
MMN = 512                  # matmul moving free dim (one PSUM bank, f32)

# in-DMA chunk schedule: small first chunks prime the pipeline, then big
# contiguous transfers (8 KiB per-partition lines) for DMA efficiency
IN_SIZES = [512, 512, 1024, 2048, 4096, 4096, 4096]
assert sum(IN_SIZES) == FTOK

# compute block schedule: small blocks prime the pipeline and shrink the
# final drain -> out-DMA tail; big middle blocks amortize instruction
# overheads (1 LDWEIGHTS + 1 multi-bank matmul per weight per block)
BLK_SIZES = [512, 512, 1024] + [2048] * 6 + [1024, 512, 512]
assert sum(BLK_SIZES) == FTOK

_NC_CACHE = {}


def _build_nc():
    import concourse.bacc as bacc
    import concourse.mybir as mybir
    import concourse.tile as tile

    f16 = mybir.dt.float16
    f32 = mybir.dt.float32

    nc = bacc.Bacc()
    # tbl = [A (D) | B (D) | cos (NSH) | sin (NSH)] fused
    TBL = 2 * D + 2 * NSH
    xin = nc.declare_dram_parameter("xin", [D, FTOK], f16, isOutput=False)
    tbl = nc.declare_dram_parameter("tbl", [D, TBL], f16, isOutput=False)
    out = nc.declare_dram_parameter("out", [D, FTOK], f16, isOutput=True)

    in_chunks = []
    off = 0
    for s in IN_SIZES:
        in_chunks.append((off, s))
        off += s

    nblk = FTOK // BLK

    PSB = 1024  # PSUM tile columns (2 banks); bufs=4 -> all 8 banks, deep pipe

    with tile.TileContext(nc) as tc:
        with (
            tc.tile_pool(name="consts", bufs=1) as consts,
            tc.tile_pool(name="inp", bufs=1) as inp,
            tc.tile_pool(name="xcs", bufs=6) as xcsp,
            tc.tile_pool(name="outp", bufs=6) as outp,
            tc.tile_pool(name="pp", bufs=4, space="PSUM") as pp,
        ):
            # weights and trig in SEPARATE tiles so their readers don't
            # false-depend on each other's DMAs (tile-granular tracking)
            wab_t = consts.tile([D, 2 * D], f16, tag="wab", name="wab_t")
            trig_t = consts.tile([D, 2 * NSH], f16, tag="trig", name="trig_t")
            nc.sync.dma_start(out=trig_t, in_=tbl[:, 2 * D :])
            a_t = wab_t[:, 0:D]
            b_t = wab_t[:, D : 2 * D]
            # trig table as [p][two][n]: cos at two=0, sin at two=1
            trig = trig_t.rearrange("p (two n) -> p two n", n=NSH)

            # the entire input stream, dispatched up-front: no reuse, no
            # stalls. The first two chunks go out the (idle) ACT HWDGE queue
            # in parallel with the trig dispatch on SP, landing earlier.
            x_tiles = []
            for i, (off, size) in enumerate(in_chunks):
                x = inp.tile([D, size], f16, tag=f"x{off}", name=f"x{off}")
                eng = nc.scalar if i < 2 else nc.sync
                eng.dma_start(out=x, in_=xin[:, off : off + size])
                x_tiles.append((off, size, x))
                if i == 0:
                    nc.sync.dma_start(out=wab_t, in_=tbl[:, 0 : 2 * D])

            def tt_fused(xcs, bs, src_tile, src_off, c0):
                """xcs[:, 0, :] = src*cos, xcs[:, 1, :] = src*sin in ONE DVE
                op (halves per-op overhead; 2x mode preserved: broadcast/two
                dims are middle dims, last dim stays stride-1). Blocks are
                512-aligned and contained in one trig half and one chunk, so
                the table slice always starts at 0 with period 512.
                """
                x3 = xcs.rearrange("p (two b) -> p two b", b=bs)
                s_sl = src_tile[:, src_off : src_off + bs]
                if bs <= NSH:
                    t0 = c0 % NSH
                    nc.vector.tensor_mul(
                        x3,
                        s_sl.unsqueeze(1).broadcast_to((D, 2, bs)),
                        trig[:, :, t0 : t0 + bs],
                    )
                else:
                    r = bs // NSH
                    nc.vector.tensor_mul(
                        x3.rearrange("p two (r n) -> p two r n", n=NSH),
                        s_sl.rearrange("p (r n) -> p r n", n=NSH)
                        .unsqueeze(1)
                        .broadcast_to((D, 2, r, NSH)),
                        trig.unsqueeze(2).broadcast_to((D, 2, r, NSH)),
                    )

            def mm(ps, w, src, size, start, stop):
                # matmul ISA caps the moving free dim at 512 (one PSUM bank)
                for h in range(0, size, MMN):
                    sl = slice(h, h + min(MMN, size - h))
                    nc.tensor.matmul(ps[:, sl], w, src[:, sl], start=start, stop=stop)

            # out-DMA grouping: small compute blocks share a 2048-col ot tile
            # and go out as one DMA (packet efficiency); the last groups stay
            # small to shrink the final drain -> out-DMA tail
            OUT_GROUPS = [2048] * 7 + [1024, 512, 512]
            assert sum(OUT_GROUPS) == FTOK
            grp_bounds = []
            g0 = 0
            for gs in OUT_GROUPS:
                grp_bounds.append((g0, gs))
                g0 += gs

            c0 = 0
            gi = 0
            ot = None
            for bs in BLK_SIZES:
                xcs = xcsp.tile([D, 2 * bs], f16, tag="xcs", name="xcs")
                for off, size, x in x_tiles:
                    if off <= c0 and c0 + bs <= off + size:
                        tt_fused(xcs, bs, x, c0 - off, c0)
                        break
                else:
                    raise AssertionError("block not contained in one chunk")
                xc = xcs[:, 0:bs]
                xs = xcs[:, bs : 2 * bs]

                g0, gs = grp_bounds[gi]
                if ot is None:
                    ot = outp.tile([D, gs], f16, tag="ot", name="ot")
                for p0 in range(0, bs, PSB):
                    sz = min(PSB, bs - p0)
                    ps = pp.tile([D, sz], f32, tag="ps", name="ps")
                    mm(ps, a_t, xc[:, p0 : p0 + sz], sz, True, False)
                    mm(ps, b_t, xs[:, p0 : p0 + sz], sz, False, True)
                    # PSUM->SBUF drain on ACT (GPSIMD can't read PSUM); the
                    # second-to-last block drains on DVE so the three tail
                    # drain->dispatch chains run on different engines instead
                    # of serializing on ACT
                    osl = ot[:, c0 - g0 + p0 : c0 - g0 + p0 + sz]
                    if gi == len(grp_bounds) - 2:
                        nc.vector.tensor_copy(out=osl, in_=ps)
                    else:
                        nc.scalar.copy(out=osl, in_=ps)
                c0 += bs
                if c0 == g0 + gs:
                    # GPSIMD SWDGE queue for most groups — it runs parallel
                    # to the SP input queue and measures ~10% better DMA
                    # engine utilization than stacking outs on SP; the tail
                    # groups go from ACT right after its own drain, avoiding
                    # a cross-engine semaphore hop
                    if gi < 7:
                        nc.gpsimd.dma_start(out=out[:, g0 : g0 + gs], in_=ot)
                    elif gi == len(grp_bounds) - 2:
                        nc.sync.dma_start(out=out[:, g0 : g0 + gs], in_=ot)
                    else:
                        nc.scalar.dma_start(out=out[:, g0 : g0 + gs], in_=ot)
                    ot = None
                    gi += 1

    nc.finalize()
    return nc


def _get_nc():
    if "nc" not in _NC_CACHE:
        _NC_CACHE["nc"] = _build_nc()
    return _NC_CACHE["nc"]


def _default_freqs():
    # computed in f32 end-to-end to match the reference's jnp arithmetic
    e = np.arange(0, D, 2, dtype=np.float32) / np.float32(D)
    return (np.float32(1.0) / np.float32(10000.0) ** e).astype(np.float32)


def _default_s_params():
    # Reproduce reference.setup_inputs()'s jax PRNG stream for s_params.
    # Must run on the CPU backend: the neuron/axon lowering of the threefry
    # PRNG produces a different stream than the CPU one the reference uses.
    import jax

    cpu = jax.local_devices(backend="cpu")[0]
    with jax.default_device(cpu):
        key = jax.random.key(0)
        _, _, k3 = jax.random.split(key, 3)
        num_s = D * (D - 1) // 2
        return np.asarray(
            0.02 * jax.random.normal(k3, (num_s,), dtype="float32"),
            dtype=np.float32,
        )


def _host_prep(pos, freqs, s_params):
    """Cayley matrices (A, Bm as lhsT) and cos/sin tables, all fp16."""
    rows, cols = np.triu_indices(D, 1)
    S = np.zeros((D, D), np.float64)
    sp = np.asarray(s_params, dtype=np.float64)
    S[rows, cols] = sp
    S[cols, rows] = -sp
    I = np.eye(D)
    C = (I - S) @ np.linalg.inv(I + S)
    Bm = np.empty_like(C)
    Bm[:, 0::2] = C[:, 1::2]
    Bm[:, 1::2] = -C[:, 0::2]
    a_lhsT = np.ascontiguousarray(C.T.astype(np.float16))
    b_lhsT = np.ascontiguousarray(Bm.T.astype(np.float16))

    # angle computed in f32 to match the reference's rounding, trig in f64
    ang = np.asarray(freqs, np.float32)[:, None] * np.asarray(pos, np.float32)[None, :]
    ang64 = ang.astype(np.float64)
    cosT = np.repeat(np.cos(ang64), 2, axis=0).astype(np.float16)  # (D, N)
    sinT = np.repeat(np.sin(ang64), 2, axis=0).astype(np.float16)
    return a_lhsT, b_lhsT, cosT, sinT


LAST_RESULTS = None


def kernel(q, k, pos=None, freqs=None, s_params=None, _run_kwargs=None, **_ignored):
    q = np.asarray(q, dtype=np.float32)
    k = np.asarray(k, dtype=np.float32)
    if pos is None:
        pos = np.arange(N, dtype=np.float32)
    if freqs is None:
        freqs = _default_freqs()
    if s_params is None:
        s_params = _default_s_params()

    a_lhsT, b_lhsT, cosT, sinT = _host_prep(pos, freqs, s_params)

    q16 = q.astype(np.float16)
    k16 = k.astype(np.float16)

    in_maps = []
    for c in range(NCORES):
        ssl = slice(c * NSH, (c + 1) * NSH)
        qT = q16[:, ssl, :].reshape(TOK, D).T
        kT = k16[:, ssl, :].reshape(TOK, D).T
        blob = np.concatenate(
            [a_lhsT, b_lhsT, cosT[:, ssl], sinT[:, ssl]], axis=1
        )
        in_maps.append(
            {
                "xin": np.ascontiguousarray(np.concatenate([qT, kT], axis=1)),
                "tbl": np.ascontiguousarray(blob),
            }
        )

    from concourse.bass_utils import run_bass_kernel_spmd

    nc = _get_nc()
    res = run_bass_kernel_spmd(
        nc,
        in_maps,
        core_ids=list(range(NCORES)),
        **(_run_kwargs or {}),
    )
    global LAST_RESULTS
    LAST_RESULTS = res

    q_out = np.empty((B, N, D), np.float32)
    k_out = np.empty((B, N, D), np.float32)
    for c in range(NCORES):
        ssl = slice(c * NSH, (c + 1) * NSH)
        o = res.results[c]["out"]
        q_out[:, ssl, :] = o[:, :TOK].T.reshape(B, NSH, D).astype(np.float32)
        k_out[:, ssl, :] = o[:, TOK:].T.reshape(B, NSH, D).astype(np.float32)
    return q_out, k_out


# revision 51
# speedup vs baseline: 1.0729x; 1.0729x over previous
"""Trainium2 Bass kernel for CayleyStringPE (RoPE + Cayley orthogonal mix).

Math: out = C @ rope(x) per token, where C = (I-S)(I+S)^{-1} is a fixed
128x128 orthogonal matrix (Cayley transform of the skew-symmetric S built
from s_params), and rope applies interleaved-pair rotations by angle
pos[t]*freqs[i].

Device formulation: rope(x)_t = x_t*c_t + P x_t * s_t with P the fixed
pair-swap-sign matrix and c_t/s_t the duplicated cos/sin vectors, so

    out_t = A @ (x_t * c_t) + Bm @ (x_t * s_t),   A = C,  Bm = C @ P

i.e. two 128x128 matmuls per token tile plus two elementwise multiplies.
No cross-partition shuffles on device.

Precision: fp16 end-to-end (inputs, trig tables, weights, outputs) with
f32 PSUM accumulation.

Sharding: sequence-parallel across 8 cores (positions split 8 x 1024, all
batches on every core). cos/sin tables are per-core (128 x 1024) and reused
across the 8 batches. A/Bm replicated. No collectives.

Schedule: the whole per-core input stream (4 MiB) is brought into SBUF by
a handful of up-front DMAs with no buffer reuse, so the SP DMA queue runs
wall-to-wall with zero dependency stalls. Compute (DVE cos/sin muls, PE
matmul blocks of 2048 cols, ACT/GPSIMD PSUM drains) chases the input
stream; out-DMAs are dispatched from the ACT hardware queue and the
GPSIMD software queue so they never serialize behind queued input DMAs.
"""

import sys

import numpy as np

for _p in ("/opt/trn_rl_repo", "/opt/pypackages"):
    if _p not in sys.path:
        sys.path.insert(0, _p)

B, N, D = 8, 8192, 128
NCORES = 8
NSH = N // NCORES          # positions per core
TOK = B * NSH              # tokens per core
FTOK = 2 * TOK             # fused q|k stream columns per core
BLK = 2048                 # compute block (TT + matmul + drain granularity)
MMN = 512                  # matmul moving free dim (one PSUM bank, f32)

# in-DMA chunk schedule: small first chunks prime the pipeline, then big
# contiguous transfers (8 KiB per-partition lines) for DMA efficiency
IN_SIZES = [512, 512, 1024, 2048, 4096, 4096, 4096]
assert sum(IN_SIZES) == FTOK

# compute block schedule: small blocks prime the pipeline and shrink the
# final drain -> out-DMA tail; big middle blocks amortize instruction
# overheads (1 LDWEIGHTS + 1 multi-bank matmul per weight per block)
BLK_SIZES = [512, 512, 1024] + [2048] * 6 + [1024, 512, 512]
assert sum(BLK_SIZES) == FTOK

_NC_CACHE = {}


def _build_nc():
    import concourse.bacc as bacc
    import concourse.mybir as mybir
    import concourse.tile as tile

    f16 = mybir.dt.float16
    f32 = mybir.dt.float32

    nc = bacc.Bacc()
    # tbl = [A (D) | B (D) | cos (NSH) | sin (NSH)] fused
    TBL = 2 * D + 2 * NSH
    xin = nc.declare_dram_parameter("xin", [D, FTOK], f16, isOutput=False)
    tbl = nc.declare_dram_parameter("tbl", [D, TBL], f16, isOutput=False)
    out = nc.declare_dram_parameter("out", [D, FTOK], f16, isOutput=True)

    in_chunks = []
    off = 0
    for s in IN_SIZES:
        in_chunks.append((off, s))
        off += s

    nblk = FTOK // BLK

    PSB = 1024  # PSUM tile columns (2 banks); bufs=4 -> all 8 banks, deep pipe

    with tile.TileContext(nc) as tc:
        with (
            tc.tile_pool(name="consts", bufs=1) as consts,
            tc.tile_pool(name="inp", bufs=1) as inp,
            tc.tile_pool(name="xcs", bufs=6) as xcsp,
            tc.tile_pool(name="outp", bufs=6) as outp,
            tc.tile_pool(name="pp", bufs=4, space="PSUM") as pp,
        ):
            # weights and trig in SEPARATE tiles so their readers don't
            # false-depend on each other's DMAs (tile-granular tracking)
            wab_t = consts.tile([D, 2 * D], f16, tag="wab", name="wab_t")
            trig_t = consts.tile([D, 2 * NSH], f16, tag="trig", name="trig_t")
            nc.sync.dma_start(out=trig_t, in_=tbl[:, 2 * D :])
            a_t = wab_t[:, 0:D]
            b_t = wab_t[:, D : 2 * D]
            # trig table as [p][two][n]: cos at two=0, sin at two=1
            trig = trig_t.rearrange("p (two n) -> p two n", n=NSH)

            # the entire input stream, dispatched up-front: no reuse, no
            # stalls. The first two chunks go out the (idle) ACT HWDGE queue
            # in parallel with the trig dispatch on SP, landing earlier.
            x_tiles = []
            for i, (off, size) in enumerate(in_chunks):
                x = inp.tile([D, size], f16, tag=f"x{off}", name=f"x{off}")
                # first three chunks ride the idle ACT HWDGE queue, landing
                # in parallel with the 512KB trig load on the SP queue
                eng = nc.scalar if i < 3 else nc.sync
                eng.dma_start(out=x, in_=xin[:, off : off + size])
                x_tiles.append((off, size, x))
                if i == 0:
                    nc.sync.dma_start(out=wab_t, in_=tbl[:, 0 : 2 * D])

            def tt_fused(xcs, bs, src_tile, src_off, c0):
                """xcs[:, 0, :] = src*cos, xcs[:, 1, :] = src*sin in ONE DVE
                op (halves per-op overhead; 2x mode preserved: broadcast/two
                dims are middle dims, last dim stays stride-1). Blocks are
                512-aligned and contained in one trig half and one chunk, so
                the table slice always starts at 0 with period 512.
                """
                x3 = xcs.rearrange("p (two b) -> p two b", b=bs)
                s_sl = src_tile[:, src_off : src_off + bs]
                if bs <= NSH:
                    t0 = c0 % NSH
                    nc.vector.tensor_mul(
                        x3,
                        s_sl.unsqueeze(1).broadcast_to((D, 2, bs)),
                        trig[:, :, t0 : t0 + bs],
                    )
                else:
                    r = bs // NSH
                    nc.vector.tensor_mul(
                        x3.rearrange("p two (r n) -> p two r n", n=NSH),
                        s_sl.rearrange("p (r n) -> p r n", n=NSH)
                        .unsqueeze(1)
                        .broadcast_to((D, 2, r, NSH)),
                        trig.unsqueeze(2).broadcast_to((D, 2, r, NSH)),
                    )

            def mm(ps, w, src, size, start, stop):
                # matmul ISA caps the moving free dim at 512 (one PSUM bank)
                for h in range(0, size, MMN):
                    sl = slice(h, h + min(MMN, size - h))
                    nc.tensor.matmul(ps[:, sl], w, src[:, sl], start=start, stop=stop)

            # out-DMA grouping: small compute blocks share a 2048-col ot tile
            # and go out as one DMA (packet efficiency); the last groups stay
            # small to shrink the final drain -> out-DMA tail
            OUT_GROUPS = [2048] * 7 + [1024, 512, 512]
            assert sum(OUT_GROUPS) == FTOK
            grp_bounds = []
            g0 = 0
            for gs in OUT_GROUPS:
                grp_bounds.append((g0, gs))
                g0 += gs

            c0 = 0
            gi = 0
            ot = None
            for bs in BLK_SIZES:
                xcs = xcsp.tile([D, 2 * bs], f16, tag="xcs", name="xcs")
                for off, size, x in x_tiles:
                    if off <= c0 and c0 + bs <= off + size:
                        tt_fused(xcs, bs, x, c0 - off, c0)
                        break
                else:
                    raise AssertionError("block not contained in one chunk")
                xc = xcs[:, 0:bs]
                xs = xcs[:, bs : 2 * bs]

                g0, gs = grp_bounds[gi]
                if ot is None:
                    ot = outp.tile([D, gs], f16, tag="ot", name="ot")
                for p0 in range(0, bs, PSB):
                    sz = min(PSB, bs - p0)
                    ps = pp.tile([D, sz], f32, tag="ps", name="ps")
                    mm(ps, a_t, xc[:, p0 : p0 + sz], sz, True, False)
                    mm(ps, b_t, xs[:, p0 : p0 + sz], sz, False, True)
                    # PSUM->SBUF drain on ACT (GPSIMD can't read PSUM); the
                    # second-to-last block drains on DVE so the three tail
                    # drain->dispatch chains run on different engines instead
                    # of serializing on ACT
                    osl = ot[:, c0 - g0 + p0 : c0 - g0 + p0 + sz]
                    if gi == len(grp_bounds) - 2:
                        nc.vector.tensor_copy(out=osl, in_=ps)
                    else:
                        nc.scalar.copy(out=osl, in_=ps)
                c0 += bs
                if c0 == g0 + gs:
                    # GPSIMD SWDGE queue for most groups — it runs parallel
                    # to the SP input queue and measures ~10% better DMA
                    # engine utilization than stacking outs on SP; the tail
                    # groups go from ACT right after its own drain, avoiding
                    # a cross-engine semaphore hop
                    if gi < 7:
                        nc.gpsimd.dma_start(out=out[:, g0 : g0 + gs], in_=ot)
                    elif gi == len(grp_bounds) - 2:
                        nc.sync.dma_start(out=out[:, g0 : g0 + gs], in_=ot)
                    else:
                        nc.scalar.dma_start(out=out[:, g0 : g0 + gs], in_=ot)
                    ot = None
                    gi += 1

    nc.finalize()
    return nc


def _get_nc():
    if "nc" not in _NC_CACHE:
        _NC_CACHE["nc"] = _build_nc()
    return _NC_CACHE["nc"]


def _default_freqs():
    # computed in f32 end-to-end to match the reference's jnp arithmetic
    e = np.arange(0, D, 2, dtype=np.float32) / np.float32(D)
    return (np.float32(1.0) / np.float32(10000.0) ** e).astype(np.float32)


def _default_s_params():
    # Reproduce reference.setup_inputs()'s jax PRNG stream for s_params.
    # Must run on the CPU backend: the neuron/axon lowering of the threefry
    # PRNG produces a different stream than the CPU one the reference uses.
    import jax

    cpu = jax.local_devices(backend="cpu")[0]
    with jax.default_device(cpu):
        key = jax.random.key(0)
        _, _, k3 = jax.random.split(key, 3)
        num_s = D * (D - 1) // 2
        return np.asarray(
            0.02 * jax.random.normal(k3, (num_s,), dtype="float32"),
            dtype=np.float32,
        )


def _host_prep(pos, freqs, s_params):
    """Cayley matrices (A, Bm as lhsT) and cos/sin tables, all fp16."""
    rows, cols = np.triu_indices(D, 1)
    S = np.zeros((D, D), np.float64)
    sp = np.asarray(s_params, dtype=np.float64)
    S[rows, cols] = sp
    S[cols, rows] = -sp
    I = np.eye(D)
    C = (I - S) @ np.linalg.inv(I + S)
    Bm = np.empty_like(C)
    Bm[:, 0::2] = C[:, 1::2]
    Bm[:, 1::2] = -C[:, 0::2]
    a_lhsT = np.ascontiguousarray(C.T.astype(np.float16))
    b_lhsT = np.ascontiguousarray(Bm.T.astype(np.float16))

    # angle computed in f32 to match the reference's rounding, trig in f64
    ang = np.asarray(freqs, np.float32)[:, None] * np.asarray(pos, np.float32)[None, :]
    ang64 = ang.astype(np.float64)
    cosT = np.repeat(np.cos(ang64), 2, axis=0).astype(np.float16)  # (D, N)
    sinT = np.repeat(np.sin(ang64), 2, axis=0).astype(np.float16)
    return a_lhsT, b_lhsT, cosT, sinT


LAST_RESULTS = None


def kernel(q, k, pos=None, freqs=None, s_params=None, _run_kwargs=None, **_ignored):
    q = np.asarray(q, dtype=np.float32)
    k = np.asarray(k, dtype=np.float32)
    if pos is None:
        pos = np.arange(N, dtype=np.float32)
    if freqs is None:
        freqs = _default_freqs()
    if s_params is None:
        s_params = _default_s_params()

    a_lhsT, b_lhsT, cosT, sinT = _host_prep(pos, freqs, s_params)

    q16 = q.astype(np.float16)
    k16 = k.astype(np.float16)

    in_maps = []
    for c in range(NCORES):
        ssl = slice(c * NSH, (c + 1) * NSH)
        qT = q16[:, ssl, :].reshape(TOK, D).T
        kT = k16[:, ssl, :].reshape(TOK, D).T
        blob = np.concatenate(
            [a_lhsT, b_lhsT, cosT[:, ssl], sinT[:, ssl]], axis=1
        )
        in_maps.append(
            {
                "xin": np.ascontiguousarray(np.concatenate([qT, kT], axis=1)),
                "tbl": np.ascontiguousarray(blob),
            }
        )

    from concourse.bass_utils import run_bass_kernel_spmd

    nc = _get_nc()
    res = run_bass_kernel_spmd(
        nc,
        in_maps,
        core_ids=list(range(NCORES)),
        **(_run_kwargs or {}),
    )
    global LAST_RESULTS
    LAST_RESULTS = res

    q_out = np.empty((B, N, D), np.float32)
    k_out = np.empty((B, N, D), np.float32)
    for c in range(NCORES):
        ssl = slice(c * NSH, (c + 1) * NSH)
        o = res.results[c]["out"]
        q_out[:, ssl, :] = o[:, :TOK].T.reshape(B, NSH, D).astype(np.float32)
        k_out[:, ssl, :] = o[:, TOK:].T.reshape(B, NSH, D).astype(np.float32)
    return q_out, k_out


# revision 52
# speedup vs baseline: 1.1385x; 1.0612x over previous
"""Trainium2 Bass kernel for CayleyStringPE (RoPE + Cayley orthogonal mix).

Math: out = C @ rope(x) per token, where C = (I-S)(I+S)^{-1} is a fixed
128x128 orthogonal matrix (Cayley transform of the skew-symmetric S built
from s_params), and rope applies interleaved-pair rotations by angle
pos[t]*freqs[i].

Device formulation: rope(x)_t = x_t*c_t + P x_t * s_t with P the fixed
pair-swap-sign matrix and c_t/s_t the duplicated cos/sin vectors, so

    out_t = A @ (x_t * c_t) + Bm @ (x_t * s_t),   A = C,  Bm = C @ P

i.e. two 128x128 matmuls per token tile plus two elementwise multiplies.
No cross-partition shuffles on device.

Precision: fp16 end-to-end (inputs, trig tables, weights, outputs) with
f32 PSUM accumulation.

Sharding: sequence-parallel across 8 cores (positions split 8 x 1024, all
batches on every core). cos/sin tables are per-core (128 x 1024) and reused
across the 8 batches. A/Bm replicated. No collectives.

Schedule: the whole per-core input stream (4 MiB) is brought into SBUF by
a handful of up-front DMAs with no buffer reuse, so the SP DMA queue runs
wall-to-wall with zero dependency stalls. Compute (DVE cos/sin muls, PE
matmul blocks of 2048 cols, ACT/GPSIMD PSUM drains) chases the input
stream; out-DMAs are dispatched from the ACT hardware queue and the
GPSIMD software queue so they never serialize behind queued input DMAs.
"""

import sys

import numpy as np

for _p in ("/opt/trn_rl_repo", "/opt/pypackages"):
    if _p not in sys.path:
        sys.path.insert(0, _p)

B, N, D = 8, 8192, 128
NCORES = 8
NSH = N // NCORES          # positions per core
TOK = B * NSH              # tokens per core
FTOK = 2 * TOK             # fused q|k stream columns per core
BLK = 2048                 # compute block (TT + matmul + drain granularity)
MMN = 512                  # matmul moving free dim (one PSUM bank, f32)

# in-DMA chunk schedule: small first chunks prime the pipeline, then big
# contiguous transfers (8 KiB per-partition lines) for DMA efficiency
IN_SIZES = [512, 512, 1024, 2048, 4096, 4096, 4096]
assert sum(IN_SIZES) == FTOK

# compute block schedule: small blocks prime the pipeline and shrink the
# final drain -> out-DMA tail; big middle blocks amortize instruction
# overheads (1 LDWEIGHTS + 1 multi-bank matmul per weight per block)
BLK_SIZES = [512, 512, 1024] + [2048] * 6 + [1024, 512, 512]
assert sum(BLK_SIZES) == FTOK

_NC_CACHE = {}


def _build_nc():
    import concourse.bacc as bacc
    import concourse.mybir as mybir
    import concourse.tile as tile

    f16 = mybir.dt.float16
    f32 = mybir.dt.float32

    nc = bacc.Bacc()
    # tbl = [A (D) | B (D) | cos (NSH) | sin (NSH)] fused
    TBL = 2 * D + 2 * NSH
    xin = nc.declare_dram_parameter("xin", [D, FTOK], f16, isOutput=False)
    tbl = nc.declare_dram_parameter("tbl", [D, TBL], f16, isOutput=False)
    out = nc.declare_dram_parameter("out", [D, FTOK], f16, isOutput=True)

    in_chunks = []
    off = 0
    for s in IN_SIZES:
        in_chunks.append((off, s))
        off += s

    nblk = FTOK // BLK

    PSB = 1024  # PSUM tile columns (2 banks); bufs=4 -> all 8 banks, deep pipe

    with tile.TileContext(nc) as tc:
        with (
            tc.tile_pool(name="consts", bufs=1) as consts,
            tc.tile_pool(name="inp", bufs=1) as inp,
            tc.tile_pool(name="xcs", bufs=6) as xcsp,
            tc.tile_pool(name="outp", bufs=6) as outp,
            tc.tile_pool(name="pp", bufs=4, space="PSUM") as pp,
        ):
            # weights and trig in SEPARATE tiles so their readers don't
            # false-depend on each other's DMAs (tile-granular tracking)
            wab_t = consts.tile([D, 2 * D], f16, tag="wab", name="wab_t")
            trig_t = consts.tile([D, 2 * NSH], f16, tag="trig", name="trig_t")
            nc.sync.dma_start(out=trig_t, in_=tbl[:, 2 * D :])
            a_t = wab_t[:, 0:D]
            b_t = wab_t[:, D : 2 * D]
            # trig table as [p][two][n]: cos at two=0, sin at two=1
            trig = trig_t.rearrange("p (two n) -> p two n", n=NSH)

            # the entire input stream, dispatched up-front: no reuse, no
            # stalls. The first two chunks go out the (idle) ACT HWDGE queue
            # in parallel with the trig dispatch on SP, landing earlier.
            x_tiles = []
            for i, (off, size) in enumerate(in_chunks):
                x = inp.tile([D, size], f16, tag=f"x{off}", name=f"x{off}")
                eng = nc.scalar if i < 2 else nc.sync
                eng.dma_start(out=x, in_=xin[:, off : off + size])
                x_tiles.append((off, size, x))
                if i == 0:
                    nc.sync.dma_start(out=wab_t, in_=tbl[:, 0 : 2 * D])

            def tt_fused(xcs, bs, src_tile, src_off, c0):
                """xcs[:, 0, :] = src*cos, xcs[:, 1, :] = src*sin in ONE DVE
                op (halves per-op overhead; 2x mode preserved: broadcast/two
                dims are middle dims, last dim stays stride-1). Blocks are
                512-aligned and contained in one trig half and one chunk, so
                the table slice always starts at 0 with period 512.
                """
                x3 = xcs.rearrange("p (two b) -> p two b", b=bs)
                s_sl = src_tile[:, src_off : src_off + bs]
                if bs <= NSH:
                    t0 = c0 % NSH
                    nc.vector.tensor_mul(
                        x3,
                        s_sl.unsqueeze(1).broadcast_to((D, 2, bs)),
                        trig[:, :, t0 : t0 + bs],
                    )
                else:
                    r = bs // NSH
                    nc.vector.tensor_mul(
                        x3.rearrange("p two (r n) -> p two r n", n=NSH),
                        s_sl.rearrange("p (r n) -> p r n", n=NSH)
                        .unsqueeze(1)
                        .broadcast_to((D, 2, r, NSH)),
                        trig.unsqueeze(2).broadcast_to((D, 2, r, NSH)),
                    )

            def mm(ps, w, src, size, start, stop):
                # matmul ISA caps the moving free dim at 512 (one PSUM bank)
                for h in range(0, size, MMN):
                    sl = slice(h, h + min(MMN, size - h))
                    nc.tensor.matmul(ps[:, sl], w, src[:, sl], start=start, stop=stop)

            # out-DMA grouping: small compute blocks share a 2048-col ot tile
            # and go out as one DMA (packet efficiency); the last groups stay
            # small to shrink the final drain -> out-DMA tail
            OUT_GROUPS = [2048] * 7 + [1024, 512, 512]
            assert sum(OUT_GROUPS) == FTOK
            grp_bounds = []
            g0 = 0
            for gs in OUT_GROUPS:
                grp_bounds.append((g0, gs))
                g0 += gs

            c0 = 0
            gi = 0
            ot = None
            for bs in BLK_SIZES:
                xcs = xcsp.tile([D, 2 * bs], f16, tag="xcs", name="xcs")
                for off, size, x in x_tiles:
                    if off <= c0 and c0 + bs <= off + size:
                        tt_fused(xcs, bs, x, c0 - off, c0)
                        break
                else:
                    raise AssertionError("block not contained in one chunk")
                xc = xcs[:, 0:bs]
                xs = xcs[:, bs : 2 * bs]

                g0, gs = grp_bounds[gi]
                if ot is None:
                    ot = outp.tile([D, gs], f16, tag="ot", name="ot")
                for p0 in range(0, bs, PSB):
                    sz = min(PSB, bs - p0)
                    ps = pp.tile([D, sz], f32, tag="ps", name="ps")
                    mm(ps, a_t, xc[:, p0 : p0 + sz], sz, True, False)
                    mm(ps, b_t, xs[:, p0 : p0 + sz], sz, False, True)
                    # PSUM->SBUF drain on ACT (GPSIMD can't read PSUM); the
                    # second-to-last block drains on DVE so the three tail
                    # drain->dispatch chains run on different engines instead
                    # of serializing on ACT
                    osl = ot[:, c0 - g0 + p0 : c0 - g0 + p0 + sz]
                    if gi == len(grp_bounds) - 2:
                        nc.vector.tensor_copy(out=osl, in_=ps)
                    else:
                        nc.scalar.copy(out=osl, in_=ps)
                c0 += bs
                if c0 == g0 + gs:
                    # GPSIMD SWDGE queue for most groups — it runs parallel
                    # to the SP input queue and measures ~10% better DMA
                    # engine utilization than stacking outs on SP; the tail
                    # groups go from ACT right after its own drain, avoiding
                    # a cross-engine semaphore hop
                    if gi < 7:
                        nc.gpsimd.dma_start(out=out[:, g0 : g0 + gs], in_=ot)
                    elif gi == len(grp_bounds) - 2:
                        nc.sync.dma_start(out=out[:, g0 : g0 + gs], in_=ot)
                    else:
                        nc.scalar.dma_start(out=out[:, g0 : g0 + gs], in_=ot)
                    ot = None
                    gi += 1

    nc.finalize()
    return nc


def _get_nc():
    if "nc" not in _NC_CACHE:
        _NC_CACHE["nc"] = _build_nc()
    return _NC_CACHE["nc"]


def _default_freqs():
    # computed in f32 end-to-end to match the reference's jnp arithmetic
    e = np.arange(0, D, 2, dtype=np.float32) / np.float32(D)
    return (np.float32(1.0) / np.float32(10000.0) ** e).astype(np.float32)


def _default_s_params():
    # Reproduce reference.setup_inputs()'s jax PRNG stream for s_params.
    # Must run on the CPU backend: the neuron/axon lowering of the threefry
    # PRNG produces a different stream than the CPU one the reference uses.
    import jax

    cpu = jax.local_devices(backend="cpu")[0]
    with jax.default_device(cpu):
        key = jax.random.key(0)
        _, _, k3 = jax.random.split(key, 3)
        num_s = D * (D - 1) // 2
        return np.asarray(
            0.02 * jax.random.normal(k3, (num_s,), dtype="float32"),
            dtype=np.float32,
        )


def _host_prep(pos, freqs, s_params):
    """Cayley matrices (A, Bm as lhsT) and cos/sin tables, all fp16."""
    rows, cols = np.triu_indices(D, 1)
    S = np.zeros((D, D), np.float64)
    sp = np.asarray(s_params, dtype=np.float64)
    S[rows, cols] = sp
    S[cols, rows] = -sp
    I = np.eye(D)
    C = (I - S) @ np.linalg.inv(I + S)
    Bm = np.empty_like(C)
    Bm[:, 0::2] = C[:, 1::2]
    Bm[:, 1::2] = -C[:, 0::2]
    a_lhsT = np.ascontiguousarray(C.T.astype(np.float16))
    b_lhsT = np.ascontiguousarray(Bm.T.astype(np.float16))

    # angle computed in f32 to match the reference's rounding, trig in f64
    ang = np.asarray(freqs, np.float32)[:, None] * np.asarray(pos, np.float32)[None, :]
    ang64 = ang.astype(np.float64)
    cosT = np.repeat(np.cos(ang64), 2, axis=0).astype(np.float16)  # (D, N)
    sinT = np.repeat(np.sin(ang64), 2, axis=0).astype(np.float16)
    return a_lhsT, b_lhsT, cosT, sinT


LAST_RESULTS = None


def kernel(q, k, pos=None, freqs=None, s_params=None, _run_kwargs=None, **_ignored):
    q = np.asarray(q, dtype=np.float32)
    k = np.asarray(k, dtype=np.float32)
    if pos is None:
        pos = np.arange(N, dtype=np.float32)
    if freqs is None:
        freqs = _default_freqs()
    if s_params is None:
        s_params = _default_s_params()

    a_lhsT, b_lhsT, cosT, sinT = _host_prep(pos, freqs, s_params)

    q16 = q.astype(np.float16)
    k16 = k.astype(np.float16)

    in_maps = []
    for c in range(NCORES):
        ssl = slice(c * NSH, (c + 1) * NSH)
        qT = q16[:, ssl, :].reshape(TOK, D).T
        kT = k16[:, ssl, :].reshape(TOK, D).T
        blob = np.concatenate(
            [a_lhsT, b_lhsT, cosT[:, ssl], sinT[:, ssl]], axis=1
        )
        in_maps.append(
            {
                "xin": np.ascontiguousarray(np.concatenate([qT, kT], axis=1)),
                "tbl": np.ascontiguousarray(blob),
            }
        )

    from concourse.bass_utils import run_bass_kernel_spmd

    nc = _get_nc()
    res = run_bass_kernel_spmd(
        nc,
        in_maps,
        core_ids=list(range(NCORES)),
        **(_run_kwargs or {}),
    )
    global LAST_RESULTS
    LAST_RESULTS = res

    q_out = np.empty((B, N, D), np.float32)
    k_out = np.empty((B, N, D), np.float32)
    for c in range(NCORES):
        ssl = slice(c * NSH, (c + 1) * NSH)
        o = res.results[c]["out"]
        q_out[:, ssl, :] = o[:, :TOK].T.reshape(B, NSH, D).astype(np.float32)
        k_out[:, ssl, :] = o[:, TOK:].T.reshape(B, NSH, D).astype(np.float32)
    return q_out, k_out
